# revision 34
# baseline (speedup 1.0000x reference)
"""Trainium2 Bass kernel for the Bahdanau-style attention scorer:

    scores[b, t] = v . tanh(X[b, t] @ WO^T + WG @ g[b])

Shapes: inputs [64, 4096, 128] f32, g [64, 128], WO/WG [256, 128], v [1, 256].
Output: [64, 4096] f32.

Strategy (data-parallel over batch, 8 NeuronCores):
  - Host: cast X to bf16 and pre-transpose to [B, D, T] so the contraction
    dim D lands on SBUF partitions with plain contiguous DMAs (no on-device
    transpose). Precompute the tiny term2 C = g @ WG^T in f32 on host.
  - Device, [s, t] orientation: term1^T = WO^T.T @ X^T via TensorE (bf16,
    f32 PSUM accumulate); ScalarE applies tanh with the per-batch bias c_b
    fused as a per-partition activation bias (the throughput floor: 8.4M
    tanh/core at 1 elem/lane/cycle); TensorE contracts with v (lhsT = v
    column, M=1) into PSUM partitions {0,32,64,96} of a shared scores tile
    via column tiling; VectorE copies each filled scores tile out of PSUM
    once per 2048 tokens; plain DMAs dump the [128, 512] tiles to a DRAM
    scratch and the host gathers rows {0,32,64,96} (partition-strided DMA
    source patterns measurably degrade the Activation engine, so the
    gather stays on the host).
"""

import numpy as np
import ml_dtypes

import concourse.bass as bass
import concourse.mybir as mybir
import concourse.tile as tile
from concourse import bacc
from concourse.bass_utils import run_bass_kernel_spmd

B, T, D, S = 64, 4096, 128, 256
N_CORES = 8
B_PER_CORE = B // N_CORES  # 8
C = 2048  # token chunk per activation instruction (4 PSUM banks)
MM_N = 512  # matmul moving free dim (one PSUM bank of f32)
DMA_C = 1024  # input DMA piece size

_BF16 = ml_dtypes.bfloat16

_nc_cache = {}

# test.py reads this to get exec_time_ns from the traced run
LAST_RESULTS = None


def _build_bass():
    nc = bacc.Bacc("TRN2", target_bir_lowering=False)
    xt = nc.dram_tensor(
        "xt", [B_PER_CORE, D, T], mybir.dt.bfloat16, kind="ExternalInput"
    )
    wot = nc.dram_tensor("wot", [D, S], mybir.dt.bfloat16, kind="ExternalInput")
    # ct[p, h*B_PER_CORE + b] = C[b, h*128 + p]  (term2, f32)
    ct = nc.dram_tensor(
        "ct", [D, 2 * B_PER_CORE], mybir.dt.float32, kind="ExternalInput"
    )
    # vt[p, h] = v[h*128 + p]
    vt = nc.dram_tensor("vt", [D, 2], mybir.dt.bfloat16, kind="ExternalInput")
    # raw scores: one [128, MM_N] tile per 4-sub-chunk group; host gathers
    # rows {0,32,64,96} (avoids partition-strided DMA patterns on device)
    n_groups = B_PER_CORE * T // (4 * MM_N)
    out = nc.dram_tensor(
        "out", [n_groups, 128, MM_N], mybir.dt.float32, kind="ExternalOutput"
    )

    with tile.TileContext(nc) as tc:
        with (
            tc.tile_pool(name="consts", bufs=1) as consts,
            tc.tile_pool(name="xin", bufs=3) as xin_pool,
            tc.tile_pool(name="tanh", bufs=6) as tanh_pool,
            tc.tile_pool(name="orow", bufs=2) as orow_pool,
            tc.tile_pool(name="ps1", bufs=2, space="PSUM") as ps1_pool,
        ):
            # consts go via the scalar-engine HWDGE queue so the sync queue
            # starts streaming X immediately
            wot_sb = consts.tile([D, S], mybir.dt.bfloat16)
            nc.scalar.dma_start(wot_sb[:], wot[:])
            ct_sb = consts.tile([D, 2 * B_PER_CORE], mybir.dt.float32)
            nc.scalar.dma_start(ct_sb[:], ct[:])
            vt_sb = consts.tile([D, 2], mybir.dt.bfloat16)
            nc.scalar.dma_start(vt_sb[:], vt[:])

            # PE warm-up: ~2.6us of dummy matmuls while the first X chunk is
            # still in flight, so the HAM clock-gate reaches 2.4 GHz before
            # real work arrives (cold matmuls run at 1.2 GHz)
            scrap = consts.tile([128, MM_N], mybir.dt.bfloat16)
            nc.vector.memset(scrap[:], 0)
            warm_ps = ps1_pool.tile([128, MM_N], mybir.dt.float32, tag="mm1")
            for _ in range(6):
                nc.tensor.matmul(
                    warm_ps[:], scrap[:, :128], scrap[:], start=True, stop=True
                )

            for b in range(B_PER_CORE):
                x_b = xin_pool.tile([D, T], mybir.dt.bfloat16, tag="xb")
                if b == 0:
                    # smaller leading transfers so the first matmul/ACT
                    # starts as early as possible
                    for jj in range(2):
                        nc.sync.dma_start(
                            x_b[:, jj * MM_N : (jj + 1) * MM_N],
                            xt[b, :, jj * MM_N : (jj + 1) * MM_N],
                        )
                    for jj in range(1, T // DMA_C):
                        nc.sync.dma_start(
                            x_b[:, jj * DMA_C : (jj + 1) * DMA_C],
                            xt[b, :, jj * DMA_C : (jj + 1) * DMA_C],
                        )
                else:
                    for jj in range(T // DMA_C):
                        nc.sync.dma_start(
                            x_b[:, jj * DMA_C : (jj + 1) * DMA_C],
                            xt[b, :, jj * DMA_C : (jj + 1) * DMA_C],
                        )
                sc = None
                for j in range(T // C):
                    th_tiles = []
                    for h in range(2):
                        ps = ps1_pool.tile([128, C], mybir.dt.float32, tag="mm1")
                        for q in range(C // MM_N):
                            col = j * C + q * MM_N
                            nc.tensor.matmul(
                                ps[:, q * MM_N : (q + 1) * MM_N],
                                wot_sb[:, h * 128 : (h + 1) * 128],
                                x_b[:, col : col + MM_N],
                                start=True,
                                stop=True,
                            )
                        th = tanh_pool.tile([128, C], mybir.dt.bfloat16, tag="th")
                        nc.scalar.activation(
                            th[:],
                            ps[:],
                            mybir.ActivationFunctionType.Tanh,
                            bias=ct_sb[:, h * B_PER_CORE + b : h * B_PER_CORE + b + 1],
                            scale=1.0,
                        )
                        th_tiles.append(th)
                    # v-dot: sub-chunk k of the chunk goes to PSUM partition
                    # 32k of a scores tile borrowed from the mm1 pool slot
                    # rotation; one DVE copy + one DMA per chunk
                    for k in range(C // MM_N):
                        if k == 0:
                            sc = ps1_pool.tile([128, MM_N], mybir.dt.float32, tag="mm1")
                        for h in range(2):
                            nc.tensor.matmul(
                                sc[32 * k : 32 * k + 1, :],
                                vt_sb[:, h : h + 1],
                                th_tiles[h][:, k * MM_N : (k + 1) * MM_N],
                                start=(h == 0),
                                stop=(h == 1),
                                tile_position=(0, 32 * k),
                            )
                        if k == 3:
                            sc_sb = orow_pool.tile(
                                [128, MM_N], mybir.dt.float32, tag="scsb"
                            )
                            nc.vector.tensor_copy(sc_sb[:], sc[:])
                            G = b * (T // C) + j
                            nc.sync.dma_start(out[G], sc_sb[:])
    _dedup_ldweights(nc)
    nc.compile()
    return nc


def _dedup_ldweights(nc):
    """Drop an InstLdweights whose weights exactly match the still-loaded
    weights (no intervening PE weight change), so back-to-back same-weight
    matmuls can pipeline. Only removes sync-free LDWs."""
    n_removed = 0
    for blk in nc.m.functions[0].blocks:
        out = []
        last_key = None
        for inst in blk.instructions:
            if isinstance(inst, mybir.InstLdweights):
                si = inst.sync_info
                clean = not (si and (si.on_wait or si.on_update))
                key = (
                    str(inst.ins[0]),
                    str(getattr(inst, "tile_position", None)),
                    str(getattr(inst, "perf_mode", None)),
                    str(getattr(inst, "is_transpose", None)),
                )
                if clean and key == last_key:
                    n_removed += 1
                    continue
                last_key = key
            elif isinstance(inst, mybir.InstMatmult):
                pass  # matmul does not clobber loaded weights
            elif getattr(inst, "engine", None) == mybir.EngineType.PE:
                if not isinstance(inst, mybir.InstEventSemaphore):
                    last_key = None
            out.append(inst)
        blk.instructions[:] = out
    return n_removed


def kernel(inputs, g, WO, WG, v):
    global LAST_RESULTS
    inputs = np.asarray(inputs, dtype=np.float32)
    g = np.asarray(g, dtype=np.float32)
    WO = np.asarray(WO, dtype=np.float32)
    WG = np.asarray(WG, dtype=np.float32)
    v = np.asarray(v, dtype=np.float32)

    # term2 (tiny): C[b, s] = g[b] @ WG[s]^T
    C_all = g @ WG.T  # [B, S] f32

    # X^T per batch: [B, D, T], bf16, contiguous
    x_bf = inputs.astype(_BF16)
    xt_all = np.ascontiguousarray(x_bf.transpose(0, 2, 1))  # [B, D, T]

    wot_host = np.ascontiguousarray(WO.T).astype(_BF16)  # [D, S]
    vt_host = np.ascontiguousarray(v.reshape(2, 128).T).astype(_BF16)  # [128, 2]

    in_maps = []
    for c in range(N_CORES):
        Cc = C_all[c * B_PER_CORE : (c + 1) * B_PER_CORE]  # [8, 256]
        ct_host = np.ascontiguousarray(
            Cc.reshape(B_PER_CORE, 2, 128).transpose(2, 1, 0).reshape(128, 2 * B_PER_CORE)
        ).astype(np.float32)
        in_maps.append(
            {
                "xt": xt_all[c * B_PER_CORE : (c + 1) * B_PER_CORE],
                "wot": wot_host,
                "ct": ct_host,
                "vt": vt_host,
            }
        )

    if "nc" not in _nc_cache:
        _nc_cache["nc"] = _build_bass()
    nc = _nc_cache["nc"]

    res = run_bass_kernel_spmd(nc, in_maps, list(range(N_CORES)))
    LAST_RESULTS = res
    groups_per_batch = T // (4 * MM_N)
    outs = []
    for r in res.results:
        raw = r["out"]  # [B_PER_CORE * groups_per_batch, 128, MM_N]
        raw = raw.reshape(B_PER_CORE, groups_per_batch, 128, MM_N)
        # rows {0,32,64,96} hold sub-chunks k of each group
        picked = raw[:, :, ::32, :]  # [B_PER_CORE, g, 4, MM_N]
        outs.append(picked.reshape(B_PER_CORE, T))
    return np.concatenate(outs, axis=0)


# revision 35
# speedup vs baseline: 1.5380x; 1.5380x over previous
"""Trainium2 Bass kernel for the Bahdanau-style attention scorer:

    scores[b, t] = v . tanh(X[b, t] @ WO^T + WG @ g[b])

Shapes: inputs [64, 4096, 128] f32, g [64, 128], WO/WG [256, 128], v [1, 256].
Output: [64, 4096] f32.

Strategy (data-parallel over batch, 8 NeuronCores):
  - Host: cast X to bf16 and pre-transpose to [B, D, T] so the contraction
    dim D lands on SBUF partitions with plain contiguous DMAs (no on-device
    transpose). Precompute the tiny term2 C = g @ WG^T in f32 on host.
  - Device, [s, t] orientation: term1^T = WO^T.T @ X^T via TensorE (bf16,
    f32 PSUM accumulate); ScalarE applies tanh with the per-batch bias c_b
    fused as a per-partition activation bias (the throughput floor: 8.4M
    tanh/core at 1 elem/lane/cycle); TensorE contracts with v (lhsT = v
    column, M=1) into PSUM partitions {0,32,64,96} of a shared scores tile
    via column tiling; VectorE copies each filled scores tile out of PSUM
    once per 2048 tokens; plain DMAs dump the [128, 512] tiles to a DRAM
    scratch and the host gathers rows {0,32,64,96} (partition-strided DMA
    source patterns measurably degrade the Activation engine, so the
    gather stays on the host).
"""

import numpy as np
import ml_dtypes

import concourse.bass as bass
import concourse.mybir as mybir
import concourse.tile as tile
from concourse import bacc
from concourse.bass_utils import run_bass_kernel_spmd

B, T, D, S = 64, 4096, 128, 256
N_CORES = 8
B_PER_CORE = B // N_CORES  # 8
C = 1024  # token chunk per activation instruction (2 PSUM banks)
MM_N = 512  # matmul moving free dim (one PSUM bank of f32)
DMA_C = 1024  # input DMA piece size

_BF16 = ml_dtypes.bfloat16

_nc_cache = {}

# test.py reads this to get exec_time_ns from the traced run
LAST_RESULTS = None


def _build_bass():
    nc = bacc.Bacc("TRN2", target_bir_lowering=False)
    xt = nc.dram_tensor(
        "xt", [B_PER_CORE, D, T], mybir.dt.bfloat16, kind="ExternalInput"
    )
    wot = nc.dram_tensor("wot", [D, S], mybir.dt.bfloat16, kind="ExternalInput")
    # ct[p, h*B_PER_CORE + b] = C[b, h*128 + p]  (term2, f32)
    ct = nc.dram_tensor(
        "ct", [D, 2 * B_PER_CORE], mybir.dt.float32, kind="ExternalInput"
    )
    # vt[p, h] = v[h*128 + p]
    vt = nc.dram_tensor("vt", [D, 2], mybir.dt.bfloat16, kind="ExternalInput")
    # raw scores: one [128, MM_N] tile per 4-sub-chunk group; host gathers
    # rows {0,32,64,96} (avoids partition-strided DMA patterns on device)
    n_groups = B_PER_CORE * T // (4 * MM_N)
    out = nc.dram_tensor(
        "out", [n_groups, 128, MM_N], mybir.dt.float32, kind="ExternalOutput"
    )

    with tile.TileContext(nc) as tc:
        with (
            tc.tile_pool(name="consts", bufs=1) as consts,
            tc.tile_pool(name="xin", bufs=3) as xin_pool,
            tc.tile_pool(name="tanh", bufs=6) as tanh_pool,
            tc.tile_pool(name="orow", bufs=2) as orow_pool,
            tc.tile_pool(name="ps1", bufs=3, space="PSUM") as ps1_pool,
            tc.tile_pool(name="ps2", bufs=2, space="PSUM") as ps2_pool,
        ):
            # consts go via the scalar-engine HWDGE queue so the sync queue
            # starts streaming X immediately
            wot_sb = consts.tile([D, S], mybir.dt.bfloat16)
            nc.scalar.dma_start(wot_sb[:], wot[:])
            ct_sb = consts.tile([D, 2 * B_PER_CORE], mybir.dt.float32)
            nc.scalar.dma_start(ct_sb[:], ct[:])
            vt_sb = consts.tile([D, 2], mybir.dt.bfloat16)
            nc.scalar.dma_start(vt_sb[:], vt[:])

            # PE warm-up: ~2.6us of dummy matmuls while the first X chunk is
            # still in flight, so the HAM clock-gate reaches 2.4 GHz before
            # real work arrives (cold matmuls run at 1.2 GHz)
            scrap = consts.tile([128, MM_N], mybir.dt.bfloat16)
            nc.vector.memset(scrap[:], 0)
            warm_ps = ps2_pool.tile([128, MM_N], mybir.dt.float32, tag="sc")
            for _ in range(6):
                nc.tensor.matmul(
                    warm_ps[:], scrap[:, :128], scrap[:], start=True, stop=True
                )

            for b in range(B_PER_CORE):
                x_b = xin_pool.tile([D, T], mybir.dt.bfloat16, tag="xb")
                if b == 0:
                    # smaller leading transfers so the first matmul/ACT
                    # starts as early as possible
                    for jj in range(2):
                        nc.sync.dma_start(
                            x_b[:, jj * MM_N : (jj + 1) * MM_N],
                            xt[b, :, jj * MM_N : (jj + 1) * MM_N],
                        )
                    for jj in range(1, T // DMA_C):
                        nc.sync.dma_start(
                            x_b[:, jj * DMA_C : (jj + 1) * DMA_C],
                            xt[b, :, jj * DMA_C : (jj + 1) * DMA_C],
                        )
                else:
                    for jj in range(T // DMA_C):
                        nc.sync.dma_start(
                            x_b[:, jj * DMA_C : (jj + 1) * DMA_C],
                            xt[b, :, jj * DMA_C : (jj + 1) * DMA_C],
                        )
                sc = None
                for j in range(T // C):
                    th_tiles = []
                    for h in range(2):
                        ps = ps1_pool.tile([128, C], mybir.dt.float32, tag="mm1")
                        for q in range(C // MM_N):
                            col = j * C + q * MM_N
                            nc.tensor.matmul(
                                ps[:, q * MM_N : (q + 1) * MM_N],
                                wot_sb[:, h * 128 : (h + 1) * 128],
                                x_b[:, col : col + MM_N],
                                start=True,
                                stop=True,
                            )
                        th = tanh_pool.tile([128, C], mybir.dt.bfloat16, tag="th")
                        nc.scalar.activation(
                            th[:],
                            ps[:],
                            mybir.ActivationFunctionType.Tanh,
                            bias=ct_sb[:, h * B_PER_CORE + b : h * B_PER_CORE + b + 1],
                            scale=1.0,
                        )
                        th_tiles.append(th)
                    # v-dot: sub-chunk Q of the batch goes to PSUM partition
                    # 32*(Q%4) of a shared scores tile (column tiling); one
                    # DVE copy + one DMA per 4 sub-chunks
                    for q in range(C // MM_N):
                        Q = j * (C // MM_N) + q
                        k = Q % 4
                        if k == 0:
                            sc = ps2_pool.tile([128, MM_N], mybir.dt.float32, tag="sc")
                        for h in range(2):
                            nc.tensor.matmul(
                                sc[32 * k : 32 * k + 1, :],
                                vt_sb[:, h : h + 1],
                                th_tiles[h][:, q * MM_N : (q + 1) * MM_N],
                                start=(h == 0),
                                stop=(h == 1),
                                tile_position=(0, 32 * k),
                            )
                        if k == 3:
                            g = Q // 4
                            sc_sb = orow_pool.tile(
                                [128, MM_N], mybir.dt.float32, tag="scsb"
                            )
                            nc.vector.tensor_copy(sc_sb[:], sc[:])
                            G = b * (T // (4 * MM_N)) + g
                            nc.sync.dma_start(out[G], sc_sb[:])
    _dedup_ldweights(nc)
    nc.compile()
    return nc


def _dedup_ldweights(nc):
    """Drop an InstLdweights whose weights exactly match the still-loaded
    weights (no intervening PE weight change), so back-to-back same-weight
    matmuls can pipeline. Only removes sync-free LDWs."""
    n_removed = 0
    for blk in nc.m.functions[0].blocks:
        out = []
        last_key = None
        for inst in blk.instructions:
            if isinstance(inst, mybir.InstLdweights):
                si = inst.sync_info
                clean = not (si and (si.on_wait or si.on_update))
                key = (
                    str(inst.ins[0]),
                    str(getattr(inst, "tile_position", None)),
                    str(getattr(inst, "perf_mode", None)),
                    str(getattr(inst, "is_transpose", None)),
                )
                if clean and key == last_key:
                    n_removed += 1
                    continue
                last_key = key
            elif isinstance(inst, mybir.InstMatmult):
                pass  # matmul does not clobber loaded weights
            elif getattr(inst, "engine", None) == mybir.EngineType.PE:
                if not isinstance(inst, mybir.InstEventSemaphore):
                    last_key = None
            out.append(inst)
        blk.instructions[:] = out
    return n_removed


def kernel(inputs, g, WO, WG, v):
    global LAST_RESULTS
    inputs = np.asarray(inputs, dtype=np.float32)
    g = np.asarray(g, dtype=np.float32)
    WO = np.asarray(WO, dtype=np.float32)
    WG = np.asarray(WG, dtype=np.float32)
    v = np.asarray(v, dtype=np.float32)

    # term2 (tiny): C[b, s] = g[b] @ WG[s]^T
    C_all = g @ WG.T  # [B, S] f32

    # X^T per batch: [B, D, T], bf16, contiguous
    x_bf = inputs.astype(_BF16)
    xt_all = np.ascontiguousarray(x_bf.transpose(0, 2, 1))  # [B, D, T]

    wot_host = np.ascontiguousarray(WO.T).astype(_BF16)  # [D, S]
    vt_host = np.ascontiguousarray(v.reshape(2, 128).T).astype(_BF16)  # [128, 2]

    in_maps = []
    for c in range(N_CORES):
        Cc = C_all[c * B_PER_CORE : (c + 1) * B_PER_CORE]  # [8, 256]
        ct_host = np.ascontiguousarray(
            Cc.reshape(B_PER_CORE, 2, 128).transpose(2, 1, 0).reshape(128, 2 * B_PER_CORE)
        ).astype(np.float32)
        in_maps.append(
            {
                "xt": xt_all[c * B_PER_CORE : (c + 1) * B_PER_CORE],
                "wot": wot_host,
                "ct": ct_host,
                "vt": vt_host,
            }
        )

    if "nc" not in _nc_cache:
        _nc_cache["nc"] = _build_bass()
    nc = _nc_cache["nc"]

    res = run_bass_kernel_spmd(nc, in_maps, list(range(N_CORES)))
    LAST_RESULTS = res
    groups_per_batch = T // (4 * MM_N)
    outs = []
    for r in res.results:
        raw = r["out"]  # [B_PER_CORE * groups_per_batch, 128, MM_N]
        raw = raw.reshape(B_PER_CORE, groups_per_batch, 128, MM_N)
        # rows {0,32,64,96} hold sub-chunks k of each group
        picked = raw[:, :, ::32, :]  # [B_PER_CORE, g, 4, MM_N]
        outs.append(picked.reshape(B_PER_CORE, T))
    return np.concatenate(outs, axis=0)
